# revision 52
# baseline (speedup 1.0000x reference)
"""Trainium2 Bass kernel for nn_MASNET2 (structure-attention warped resampling).

Pipeline per batch:
  1. axis-max marginals of structure_att  -> x/y profiles
  2. linear-downsample 448->224, reflect-pad to 670 (normalization cancels in
     the conv ratio, so profiles stay unnormalized)
  3. 447-tap conv (plain + coordinate-weighted) via fp32 Toeplitz matmuls
     -> per-batch sampling coords
  4. separable bilinear grid-sample via two tent-weight matmul stages

Sharding: pure data-parallel, batch 64 -> 8 cores x 8.

Implementation notes:
  - image data is cast fp32->bf16 during the load DMA (gpsimd SWDGE casts
    in-flight), halving the dominant HBM read cost; output is stored bf16
    and widened host-side.
  - tent weights are generated bf16 (negated: min(|d|,1)-1 = -tent) from a
    DMA-broadcast fp32 coordinate row; both matmul stages consume the bf16
    tents as the moving operand so the PE runs full rate at N=224.
  - the two negations cancel across the two interpolation stages.
  - y-coords are stored in even/odd-interleaved order so stage-2 output
    partitions map to interleaved row pairs, making the bf16 output store
    fully coalesced (896B runs).
  - everything is pipelined per batch: marginals/conv/tents for batch b+1
    overlap the grid-sample matmuls of batch b; data tiles prefetch ~2
    batches ahead.
"""
import os
import sys

sys.path.insert(0, "/opt/trn_rl_repo")

import numpy as np
from contextlib import ExitStack

import concourse.bass as bass
import concourse.bacc as bacc
import concourse.tile as tile
from concourse import mybir, masks
from concourse.bass_utils import run_bass_kernel_spmd

F32 = mybir.dt.float32
F32R = mybir.dt.float32r
BF16 = mybir.dt.bfloat16
ALU = mybir.AluOpType
ACTF = mybir.ActivationFunctionType

SAM = 224
IN = 448
PAD = 223
GLOB = 670
KSIZE = 447
NCORES = 8
BSH = 8  # batch shard per core

_CACHE = {}

# expose the last run's results for test.py profiling
last_results = None


def _build_program(debug=False):
    nc = bacc.Bacc("TRN2", num_devices=NCORES)

    data_in = nc.dram_tensor("data", (3 * BSH, IN, IN), F32, kind="ExternalInput")
    att_in = nc.dram_tensor("att", (BSH, IN, IN), F32, kind="ExternalInput")
    # abmat[y, 0:224] = fused interp+pad+conv operator, [y, 224:448] = the
    # P-weighted variant: coords come from 4 accumulating fp32r matmuls
    abmat_in = nc.dram_tensor("abmat", (IN, IN), F32R, kind="ExternalInput")
    nbase_in = nc.dram_tensor("nbase", (112, 4), F32, kind="ExternalInput")

    out_dram = nc.dram_tensor("out", (BSH, 3, SAM, SAM), BF16, kind="ExternalOutput")
    ycst = nc.dram_tensor("ycst", (16, SAM), F32, kind="Internal")
    if debug:
        dbg_marg = nc.dram_tensor("dbg_marg", (BSH, 112, 4, 2), F32,
                                  kind="ExternalOutput")
        dbg_px = nc.dram_tensor("dbg_px", (BSH, 2, IN), F32,
                                kind="ExternalOutput")
        dbg_ycb = nc.dram_tensor("dbg_ycb", (BSH, 2, SAM), F32,
                                 kind="ExternalOutput")
        dbg_wyn = nc.dram_tensor("dbg_wyn", (BSH, 112, 4, SAM), BF16,
                                 kind="ExternalOutput")
        dbg_wxn = nc.dram_tensor("dbg_wxn", (BSH, 112, 4, SAM), BF16,
                                 kind="ExternalOutput")
        dbg_bt = nc.dram_tensor("dbg_bt", (BSH, 112, 4, SAM), BF16,
                                kind="ExternalOutput")

    with tile.TileContext(nc) as tc, ExitStack() as ctx:
        consts = ctx.enter_context(tc.tile_pool(name="consts", bufs=1))
        apool = ctx.enter_context(tc.tile_pool(name="apool", bufs=12))
        attp = ctx.enter_context(tc.tile_pool(name="attp", bufs=8))
        sigp = ctx.enter_context(tc.tile_pool(name="sigp", bufs=2))
        wpool = ctx.enter_context(tc.tile_pool(name="wpool", bufs=3))
        epool = ctx.enter_context(tc.tile_pool(name="epool", bufs=3))
        opool = ctx.enter_context(tc.tile_pool(name="opool", bufs=3))
        psA = ctx.enter_context(tc.tile_pool(name="psA", bufs=2, space="PSUM"))
        psB = ctx.enter_context(tc.tile_pool(name="psB", bufs=2, space="PSUM"))
        ps1 = ctx.enter_context(tc.tile_pool(name="ps1", bufs=2, space="PSUM"))

        ident = consts.tile([128, 128], F32)
        masks.make_identity(nc, ident[:])

        def load_data(j):
            at = apool.tile([112, 4, IN], BF16, tag="at")
            nc.gpsimd.dma_start(
                out=at, in_=data_in[j].rearrange("(cc p) x -> p cc x", p=112))
            return at

        # attention loads all up-front on the sync queue: they gate the
        # coordinate pipeline, while data tiles prefetch concurrently via
        # the Pool (SWDGE) queue
        att_tiles = {}
        for b in range(BSH):
            att_t = attp.tile([112, 4, IN], F32, tag="att_t")
            nc.sync.dma_start(
                out=att_t, in_=att_in[b].rearrange("(cc p) x -> p cc x", p=112))
            att_tiles[b] = att_t

        abm = consts.tile([112, 4, IN], F32R)
        nc.sync.dma_start(out=abm, in_=abmat_in.rearrange("(cc p) j -> p cc j", p=112))
        nbase = consts.tile([112, 4], F32)
        nc.sync.dma_start(out=nbase, in_=nbase_in[:, :])
        # hoist the activation-table load out of the first batch's tent chain
        warm = consts.tile([16, 2], BF16)
        nc.scalar.activation(out=warm, in_=ident[0:16, 0:2], func=ACTF.Abs,
                             bias=0.0, scale=1.0)

        at_tiles = {}
        for j in range(6):
            at_tiles[j] = load_data(j)

        # per-batch pipeline state carried between stages
        st = {}

        def stage_margA(b):
            att_t = att_tiles[b]
            # -------- marginals: x-fold first (it gates the PE transposes),
            # then the y-profile reduce -------
            marg = sigp.tile([112, 4, 2], F32R, tag="marg")
            m1 = sigp.tile([112, IN], F32, tag="m1")
            nc.vector.tensor_tensor(
                out=m1, in0=att_t[:, 0, :], in1=att_t[:, 1, :], op=ALU.max)
            m2 = sigp.tile([112, IN], F32, tag="m2")
            nc.vector.tensor_tensor(
                out=m2, in0=att_t[:, 2, :], in1=att_t[:, 3, :], op=ALU.max)
            nc.vector.tensor_tensor(out=m1, in0=m1, in1=m2, op=ALU.max)
            nc.vector.tensor_reduce(
                out=marg[:, :, 1:2], in_=att_t, axis=mybir.AxisListType.X,
                op=ALU.max)
            st[b] = {"marg": marg, "m1": m1}

        def stage_margB(b):
            marg, m1 = st[b]["marg"], st[b]["m1"]
            # x-profile: transpose folded rows, reduce
            mt_ps = ps1.tile([112, 4, 112], F32, tag="small")
            for xc in range(4):
                nc.tensor.transpose(
                    mt_ps[:, xc, :], m1[:, xc * 112:(xc + 1) * 112],
                    ident[0:112, 0:112])
            nc.vector.tensor_reduce(
                out=marg[:, :, 0:1], in_=mt_ps, axis=mybir.AxisListType.X,
                op=ALU.max)

        def stage_coordmm(b):
            marg = st[b]["marg"]
            # fused interp+reflect-pad+conv as 4 accumulating fp32r matmuls:
            # px_ps[ax, 0:224] = conv(m_ax), [ax, 224:448] = conv(P*m_ax)
            px_ps = ps1.tile([2, IN], F32, tag="small")
            for cc in range(4):
                nc.tensor.matmul(
                    px_ps, lhsT=marg[:, cc, :], rhs=abm[:, cc, :],
                    start=(cc == 0), stop=(cc == 3))
            st[b]["px_ps"] = px_ps

        def stage_coords(b):
            px_ps = st[b]["px_ps"]
            # coords = clip(447 * conv(P*m)/conv(m), 0, 447), rows [2, 224]
            rec = sigp.tile([2, SAM], F32, tag="rec")
            nc.vector.reciprocal(out=rec, in_=px_ps[:, 0:SAM])
            pc = sigp.tile([2, SAM], F32, tag="pc")
            nc.vector.tensor_tensor(
                out=pc, in0=px_ps[:, SAM:IN], in1=rec, op=ALU.mult)
            nc.vector.tensor_scalar(
                out=pc, in0=pc, scalar1=447.0, scalar2=0.0,
                op0=ALU.mult, op1=ALU.max)
            nc.vector.tensor_scalar(
                out=pc, in0=pc, scalar1=447.0, scalar2=None, op0=ALU.min)

            # stage coords to DRAM: row b = x coords (natural order),
            # row 8+b = y coords interleaved [evens | odds]
            nc.sync.dma_start(
                out=bass.AP(ycst, b * SAM, [[1, SAM]]), in_=pc[0:1, :])
            nc.sync.dma_start(
                out=bass.AP(ycst, (8 + b) * SAM, [[1, 112], [112, 2]]),
                in_=pc[1:2, :])

            # broadcast coords to all partitions: ycb[:, 0, :]=x, [:, 1, :]=y
            ycb = sigp.tile([112, 2, SAM], F32, tag="ycb")
            nc.sync.dma_start(
                out=ycb,
                in_=bass.AP(ycst, b * SAM, [[0, 112], [8 * SAM, 2], [1, SAM]]))
            if debug:
                nc.sync.dma_start(out=dbg_marg[b], in_=st[b]["marg"])
                nc.sync.dma_start(out=dbg_px[b], in_=pc)
                nc.sync.dma_start(out=dbg_ycb[b], in_=ycb[0:1, :, :])
            st[b]["ycb"] = ycb

        def stage_tents(b, ccs):
            ycb = st[b]["ycb"]
            # -------- negated bf16 tent weights: min(|d|,1)-1 --------------
            # wtn[:, cc, 0, :] = x-axis tents, [:, cc, 1, :] = y-axis tents,
            # both axes per Act/Pool op
            if ccs[0] == 0:
                wtn = wpool.tile([112, 4, 2, SAM], BF16, tag="wtn")
                st[b]["wtn"] = wtn
            else:
                wtn = st[b]["wtn"]
            for cc in ccs:
                absp = sigp.tile([112, 2, SAM], BF16, tag="absp")
                nc.scalar.activation(
                    out=absp, in_=ycb, func=ACTF.Abs,
                    bias=nbase[:, cc:cc + 1], scale=1.0)
                nc.gpsimd.tensor_scalar(
                    out=wtn[:, cc, :, :], in0=absp, scalar1=1.0, scalar2=1.0,
                    op0=ALU.min, op1=ALU.subtract)
            if debug and ccs[-1] == 3:
                nc.sync.dma_start(
                    out=dbg_wxn[b],
                    in_=bass.AP(wtn.tensor, wtn.offset,
                                [list(wtn[:].ap[0]), [2 * SAM, 4], [1, SAM]]))
                nc.sync.dma_start(
                    out=dbg_wyn[b],
                    in_=bass.AP(wtn.tensor, wtn.offset + SAM,
                                [list(wtn[:].ap[0]), [2 * SAM, 4], [1, SAM]]))

        bt_st = {}

        def gs_stage1(b, c):
            wtn = st[b]["wtn"]
            at = at_tiles[3 * b + c]
            # 256-padded xc stride keeps each accumulation group inside
            # one PSUM bank
            btps = psA.tile([112, 4, 256], F32, tag="btps")
            for xc in range(4):
                for yc_ in range(4):
                    nc.tensor.matmul(
                        btps[:, xc, 0:SAM],
                        lhsT=at[:, yc_, xc * 112:(xc + 1) * 112],
                        rhs=wtn[:, yc_, 1, :],
                        start=(yc_ == 0), stop=(yc_ == 3))
            bt = epool.tile([112, 4, SAM], BF16, tag="bt")
            nc.vector.tensor_copy(out=bt[:, 0:1, :], in_=btps[:, 0:1, 0:SAM])
            nc.scalar.copy(out=bt[:, 1:4, :], in_=btps[:, 1:4, 0:SAM])
            bt_st[(b, c)] = bt

        def gs_stage2(b, c):
            wtn = st[b]["wtn"]
            bt = bt_st.pop((b, c))
            osps = psB.tile([112, 2, SAM], F32, tag="osps")
            for ih in range(2):
                for xc in range(4):
                    nc.tensor.matmul(
                        osps[:, ih, :],
                        lhsT=bt[:, xc, ih * 112:(ih + 1) * 112],
                        rhs=wtn[:, xc, 0, :],
                        start=(xc == 0), stop=(xc == 3))
            if debug and c == 0:
                nc.sync.dma_start(out=dbg_bt[b], in_=bt)
            osb = opool.tile([112, 2, SAM], BF16, tag="osb")
            if c == 0:
                nc.vector.tensor_copy(out=osb, in_=osps)
            else:
                nc.scalar.copy(out=osb, in_=osps)
            nc.scalar.dma_start(
                out=out_dram[b, c].rearrange("(p ih) j -> p ih j", ih=2),
                in_=osb)

        # ---- software-pipelined emission ---------------------------------
        # batch b+2's coordinate chain and batch b+1's tent generation
        # interleave with batch b's grid-sample matmuls; grid-sample stage 2
        # trails stage 1 by one channel so the PSUM evacuation never stalls
        # the PE.
        # lead-in: batch 0's chain end-to-end first, then batches 1-2
        stage_margA(0)
        stage_margB(0)
        stage_coordmm(0)
        stage_coords(0)
        stage_tents(0, (0, 1, 2, 3))
        for b in (1, 2):
            stage_margA(b)
        stage_margB(1)
        stage_coordmm(1)
        stage_coords(1)

        from collections import deque
        pending = deque()  # (b, c) pairs awaiting stage 2, two behind

        def emit_s1(b, c):
            gs_stage1(b, c)
            pending.append((b, c))
            if len(pending) > 2:
                gs_stage2(*pending.popleft())

        for b in range(BSH):
            if b + 2 < BSH:
                stage_margB(b + 2)
                for j in range(3 * b + 6, 3 * b + 9):
                    at_tiles[j] = load_data(j)
            emit_s1(b, 0)
            if b + 3 < BSH:
                stage_margA(b + 3)
            if b + 2 < BSH:
                stage_coordmm(b + 2)
            emit_s1(b, 1)
            if b + 2 < BSH:
                stage_coords(b + 2)
            if b + 1 < BSH:
                stage_tents(b + 1, (0, 1, 2, 3))
            emit_s1(b, 2)
        while pending:
            gs_stage2(*pending.popleft())
    nc.compile()
    return nc


def _static_consts(filter_w: np.ndarray):
    # fused linear operator: marginal profile [448] -> (conv(m), conv(P*m))
    # composed from interp (448->224), reflect-pad (224->670), and the
    # 447-tap valid conv (670->224)
    fw = filter_w.astype(np.float64)
    pos = np.arange(SAM) * ((IN - 1) / (SAM - 1.0))
    i0 = np.floor(pos).astype(int)
    i1 = np.minimum(i0 + 1, IN - 1)
    w = pos - i0
    wint = np.zeros((IN, SAM))
    wint[i0, np.arange(SAM)] += 1.0 - w
    wint[i1, np.arange(SAM)] += w
    pmat = np.zeros((SAM, GLOB))
    g = np.arange(GLOB)
    mm = g - PAD
    src = np.where(mm < 0, -mm, np.where(mm > SAM - 1, 2 * (SAM - 1) - mm, mm))
    pmat[src, g] = 1.0
    toep = np.zeros((GLOB, SAM))
    for o in range(SAM):
        toep[o:o + KSIZE, o] = fw
    prow = (np.arange(GLOB) - PAD) / (SAM - 1.0)
    wp = wint @ pmat
    abmat = np.concatenate(
        [wp @ toep, wp @ (prow[:, None] * toep)], axis=1).astype(np.float32)

    base = (np.arange(112, dtype=np.float32)[:, None]
            + 112.0 * np.arange(4, dtype=np.float32)[None, :])
    nbase = (-base).astype(np.float32)
    return {"abmat": abmat, "nbase": nbase}


def kernel(data: np.ndarray, structure_att: np.ndarray,
           filter_w: np.ndarray) -> np.ndarray:
    global last_results
    data = np.ascontiguousarray(data, dtype=np.float32)
    structure_att = np.ascontiguousarray(structure_att, dtype=np.float32)
    filter_w = np.ascontiguousarray(filter_w, dtype=np.float32)

    if "nc" not in _CACHE:
        _CACHE["nc"] = _build_program()
    nc = _CACHE["nc"]

    consts = _static_consts(filter_w)
    in_maps = []
    for core in range(NCORES):
        sl = slice(core * BSH, (core + 1) * BSH)
        in_maps.append({
            "data": data[sl].reshape(3 * BSH, IN, IN),
            "att": structure_att[sl], **consts,
        })

    res = run_bass_kernel_spmd(nc, in_maps, core_ids=list(range(NCORES)))
    last_results = res
    out = np.concatenate(
        [np.asarray(res.results[i]["out"]).astype(np.float32)
         for i in range(NCORES)], axis=0)
    return out
